# revision 63
# baseline (speedup 1.0000x reference)
"""Trainium2 Bass kernel for nn_MultiHeadFast (multi-head attention with
softmax over the QUERY axis).

Math (faithful to the reference):
  qkv = x @ Ws;  per (b,h):  S[q,k] = Q.K^T,  causal mask k<=q,
  P = softmax_over_q(S * T^-0.5),  out = P @ V.

Layout strategy (v4):
  * Host passes x TRANSPOSED and in bf16: xT (E, NT); device never
    transposes x.  Q^T / K^T are computed feature-on-partition; V is
    computed directly in NATURAL layout (tokens on partitions) for the PV
    stationary.  out^T is DMA'd out and transposed on the host.
  * S is computed TRANSPOSED (S^T[k, q], keys on partitions) so the
    query-axis softmax is a free-axis reduction (ACT accum during exp).
    Strips start exactly at the 128-aligned causal diagonal; the causal
    mask of the diagonal block is seeded INTO the S PSUM group by an extra
    matmul (tri_neg^T @ I), keeping the S->exp critical path PE-only.
  * QKV work is cut into small pieces (one PSUM tile each) that are
    interleaved down the attention k-loop via a deadline schedule, so the
    ACT engine never starves behind a monolithic QKV block and the PE
    stays busy (and at full clock) through the ACT-bound phase.
  * attn(b=0) runs its k-loop DESCENDING: k=15 needs only the last 128
    tokens of QKV, so attention starts ~4us in.  attn(b=1) runs ASCENDING:
    its PSUM output banks retire one-by-one (k=4j+3), spreading the output
    DMAs and shrinking the tail.
  * exp has no max-subtraction: |S*c| < 1.5.  bf16 with fp32 accumulation.

Sharding: tensor-parallel over heads.  Core c owns heads {2c, 2c+1}; no
collectives.
"""

import numpy as np
import ml_dtypes
from contextlib import ExitStack

import concourse.bass as bass
import concourse.mybir as mybir
import concourse.tile as tile
from concourse import bacc
from concourse.bass_utils import run_bass_kernel_spmd
from concourse.masks import make_identity

B, T, E = 2, 2048, 1024
H, D = 16, 64
NCORES = 8
HPC = H // NCORES            # heads per core = 2
FPC = HPC * D                # feature cols per core per Q/K/V = 128
P = 128
NT = B * T                   # 4096 tokens total
EK = E // P                  # 8 contraction blocks for QKV
KTILES = T // P              # 16 key tiles per batch
NSLAB = T // 512             # 4 query slabs per batch
DT = mybir.dt.bfloat16
F32 = mybir.dt.float32
SCALE = float(T) ** -0.5
NEG = -1e30


def build_kernel():
    nc = bacc.Bacc("TRN2", target_bir_lowering=False, debug=False)
    xt_dram = nc.dram_tensor("xt", (E, NT), DT, kind="ExternalInput")
    w_dram = nc.dram_tensor("wsl", (E, 3 * FPC), DT, kind="ExternalInput")
    out_dram = nc.dram_tensor("out", (FPC, NT), F32, kind="ExternalOutput")

    with tile.TileContext(nc) as tc, ExitStack() as ctx:
        const = ctx.enter_context(tc.tile_pool(name="const", bufs=1))
        big = ctx.enter_context(tc.tile_pool(name="big", bufs=1))
        strips = ctx.enter_context(tc.tile_pool(name="strips", bufs=4))
        small = ctx.enter_context(tc.tile_pool(name="small", bufs=8))
        outp = ctx.enter_context(tc.tile_pool(name="outp", bufs=2))
        ps = ctx.enter_context(tc.tile_pool(name="ps", bufs=2, space="PSUM"))

        # ---- input DMAs first: every engine-queue's first work is a load,
        # so transfers overlap the constant setup below ----
        wsl = big.tile([P, EK, 3 * FPC], DT, name="wsl")
        xT = big.tile([P, EK, NT], DT, name="xT")
        xt_view = xt_dram.rearrange("(eo ei) t -> ei eo t", ei=P)
        w_view = w_dram.rearrange("(eo ei) f -> ei eo f", ei=P)
        # wsl gates the first QKV matmuls: split across three DMA queues
        nc.gpsimd.dma_start(wsl[:, 0:3, :], w_view[:, 0:3, :])
        nc.sync.dma_start(xT[:, :, 1920:2048], xt_view[:, :, 1920:2048])
        nc.scalar.dma_start(wsl[:, 6:8, :], w_view[:, 6:8, :])
        nc.sync.dma_start(wsl[:, 3:6, :], w_view[:, 3:6, :])
        nc.sync.dma_start(xT[:, :, 1536:1920], xt_view[:, :, 1536:1920])

        # ---- constants ----
        zeros_bf = const.tile([P, P], DT, name="zeros_bf")
        nc.gpsimd.memset(zeros_bf[:], 0.0)
        id_bf = const.tile([P, P], DT, name="id_bf")
        make_identity(nc, id_bf)
        # tri_neg[c, m] = NEG if c < m else 0.  Used as matmul stationary to
        # seed the causal mask into the S PSUM group on the PE itself:
        # (tri_neg^T @ ext_id)[p, f] = tri_neg[f, p] = NEG iff f < p (f<128).
        tri_neg = const.tile([P, P], DT, name="tri_neg")
        nc.gpsimd.memset(tri_neg[:], 0.0)
        nc.gpsimd.affine_select(
            out=tri_neg[:],
            in_=tri_neg[:],
            compare_op=mybir.AluOpType.is_ge,
            fill=NEG,
            base=0,
            pattern=[[-1, P]],
            channel_multiplier=1,
        )
        # ext_id = [I | 0 | 0 | 0]: a 512-wide moving operand so the mask
        # seed covers the whole first S block in ONE matmul.
        ext_id = const.tile([P, 512], DT, name="ext_id")
        nc.gpsimd.memset(ext_id[:], 0.0)
        nc.vector.tensor_copy(ext_id[:, 0:P], id_bf[:])

        # ---- persistent SBUF tensors ----
        qkt = big.tile([P, 2, NT], DT, name="qkt")   # [:,0,:]=Q^T  [:,1,:]=K^T
        v_nat = big.tile([P, B * KTILES, FPC], DT, name="v_nat")

        # ---- phase-A piece builders (each = one short PSUM-tile lifetime) --
        def pa_dma(ta, w):
            def run():
                nc.sync.dma_start(xT[:, :, ta : ta + w], xt_view[:, :, ta : ta + w])
            return run

        def pa_qk(m, ta, w):
            def run():
                qk_ps = ps.tile([P, 1024], F32, tag="sps", bufs=2, name="qk_ps")
                for e in range(EK):
                    nc.tensor.matmul(
                        qk_ps[:, 0:w],
                        lhsT=wsl[:, e, m * P : (m + 1) * P],
                        rhs=xT[:, e, ta : ta + w],
                        start=(e == 0),
                        stop=(e == EK - 1),
                    )
                nc.vector.tensor_copy(qkt[:, m, ta : ta + w], qk_ps[:, 0:w])
            return run

        def pa_v(blk, nblk):
            def run():
                v_ps = ps.tile([P, 1024], F32, tag="sps", bufs=2, name="v_ps")
                for i in range(nblk):
                    t0 = (blk + i) * P
                    for e in range(EK):
                        nc.tensor.matmul(
                            v_ps[:, i * P : (i + 1) * P],
                            lhsT=xT[:, e, t0 : t0 + P],
                            rhs=wsl[:, e, 2 * FPC : 3 * FPC],
                            start=(e == 0),
                            stop=(e == EK - 1),
                        )
                nc.vector.tensor_copy(
                    v_nat[:, blk : blk + nblk, :],
                    v_ps[:, 0 : nblk * P].rearrange("p (tb f) -> p tb f", tb=nblk),
                )
            return run

        # ---- attention helpers ----
        def s_chunk(b, k, hh, strip, coff, cw):
            """S^T matmuls (+ causal-mask PSUM seed) + exp for one <=1024-wide
            chunk of a strip."""
            q0 = P * k
            kt = qkt[hh * D : (hh + 1) * D, 1, b * T + q0 : b * T + q0 + P]
            sps = ps.tile([P, 1024], F32, tag="sps", bufs=2, name="sps")
            for so in range(0, cw, 512):
                w = min(512, cw - so)
                qs = b * T + q0 + coff + so
                first = coff == 0 and so == 0
                if first:
                    # causal-mask seed for the diagonal block, one 512-wide
                    # matmul; the S matmul then accumulates on top of it.
                    nc.tensor.matmul(
                        sps[:, 0:w],
                        lhsT=tri_neg[:],
                        rhs=ext_id[:, 0:w],
                        start=True,
                        stop=False,
                    )
                nc.tensor.matmul(
                    sps[:, so : so + w],
                    lhsT=kt,
                    rhs=qkt[hh * D : (hh + 1) * D, 0, qs : qs + w],
                    start=not first,
                    stop=True,
                )
            acc = small.tile([P, 1], F32, tag="acc", name="acc")
            nc.scalar.activation(
                strip[:, coff : coff + cw],
                sps[:, :cw],
                mybir.ActivationFunctionType.Exp,
                scale=SCALE,
                accum_out=acc[:],
            )
            return acc

        def finish_head(b, k, hh, partials):
            if len(partials) == 1:
                ssum = partials[0]
            else:
                ssum = small.tile([P, 1], F32, tag="acc", name="ssum")
                nc.vector.tensor_add(ssum[:], partials[0][:], partials[1][:])
            rsum = small.tile([P, 1], F32, tag="acc", name="rsum")
            nc.vector.reciprocal(rsum[:], ssum[:])
            vp = small.tile([P, D], DT, tag="vp", name="vp")
            nc.vector.tensor_scalar_mul(
                vp[:], v_nat[:, b * KTILES + k, hh * D : (hh + 1) * D], rsum[:]
            )
            return vp

        def evac_slab(b, j, pv_ps):
            osb = outp.tile([P, 512], F32, tag="osb", name="osb")
            nc.vector.tensor_copy(osb[:], pv_ps[j][:])
            nc.sync.dma_start(
                out_dram[:, b * T + 512 * j : b * T + 512 * (j + 1)], osb[:]
            )

        def attn_batch(b, pieces_by_k, descending):
            """Attention for batch b.  pieces_by_k maps ('pre'|'post', k) ->
            piece closures emitted at the top of / mid that iteration."""
            last_k = 0 if descending else KTILES - 1
            # zero-init each PV bank just before its first writer, off the
            # prefix critical path (descending: pv[j] first written by
            # PV(4j+3), issued during iteration 4j+2; ascending: all four
            # banks first written by PV(0), issued during iteration 1).
            zinit_at = (
                {15: [3], 12: [2], 8: [1], 4: [0]}
                if descending
                else {0: [0, 1, 2, 3]}
            )

            def pv_head(k, hh, strip, vp, pv_ps):
                q0 = P * k
                j0 = k // 4
                dead = q0 - 512 * j0
                for j in range(j0, NSLAB):
                    stop = k == (0 if descending else 4 * j + 3) and hh == HPC - 1
                    if j == j0:
                        nc.tensor.matmul(
                            pv_ps[j][hh * D : (hh + 1) * D, dead:512],
                            lhsT=vp[:],
                            rhs=strip[:, 0 : 512 - dead],
                            start=False,
                            stop=stop,
                            skip_group_check=True,
                        )
                    else:
                        nc.tensor.matmul(
                            pv_ps[j][hh * D : (hh + 1) * D, :],
                            lhsT=vp[:],
                            rhs=strip[:, 512 * j - q0 : 512 * j - q0 + 512],
                            start=False,
                            stop=stop,
                            skip_group_check=True,
                        )

            pv_ps = [
                ps.tile([P, 512], F32, tag="pv", bufs=4, name=f"pv_{b}_{j}")
                for j in range(NSLAB)
            ]

            def zinit(j):
                # rhs must be initialized data (0 * NaN = NaN): wsl is loaded
                # before anything else
                nc.tensor.matmul(
                    pv_ps[j][:],
                    lhsT=zeros_bf[:],
                    rhs=wsl.rearrange("p e f -> p (e f)")[:, 0:512],
                    start=True,
                    stop=False,
                    skip_group_check=True,
                )

            ks = range(KTILES - 1, -1, -1) if descending else range(KTILES)
            prev = {}
            prev_k = None
            for k in ks:
                for piece in pieces_by_k.get(("pre", k), ()):
                    piece()
                L = T - P * k
                strip_k = {}
                parts = {0: [], 1: []}
                for hh in range(HPC):
                    strip_k[hh] = strips.tile([P, T], DT, tag="strip", name=f"s{hh}")
                coff = 0
                while coff < L:
                    cw = min(1024, L - coff)
                    for hh in range(HPC):
                        parts[hh].append(s_chunk(b, k, hh, strip_k[hh], coff, cw))
                    coff += cw
                # post-slot pieces: fill the PE while exp(k) streams on ACT
                for j in zinit_at.get(k, ()):
                    zinit(j)
                for piece in pieces_by_k.get(("post", k), ()):
                    piece()
                for hh in range(HPC):
                    vp = finish_head(b, k, hh, parts[hh])
                    if prev_k is not None:
                        pv_head(prev_k, hh, *prev[hh], pv_ps)
                    prev[hh] = (strip_k[hh], vp)
                if prev_k is not None and not descending and prev_k % 4 == 3:
                    evac_slab(b, prev_k // 4, pv_ps)
                prev_k = k
            for hh in range(HPC):
                pv_head(last_k, hh, *prev[hh], pv_ps)
            if descending:
                for j in range(NSLAB):
                    evac_slab(b, j, pv_ps)
            else:
                evac_slab(b, NSLAB - 1, pv_ps)

        # ---- program order ----
        # Short PE warmup (fits inside the initial DMA wait): ramps the PE
        # p-state so the first real QKV pieces run near full clock.
        warm_ps = ps.tile([P, 1024], F32, tag="sps", bufs=2, name="warm")
        for i in range(6):
            nc.tensor.matmul(
                warm_ps[:, 0:512],
                lhsT=zeros_bf[:],
                rhs=ext_id[:],
                start=True,
                stop=True,
                skip_group_check=True,
            )
        # prefix: just enough of slab 3 (tokens 1920-2048) for attn0's k=15
        # (its DMAs were issued at the very top)
        pa_qk(0, 1920, 128)()
        pa_qk(1, 1920, 128)()
        # attn0 runs k DESCENDING; QKV pieces stream in by deadline.
        attn_batch(
            0,
            {
                ("post", 15): [pa_v(15, 1), pa_qk(0, 1536, 384),
                               pa_qk(1, 1536, 384)],
                ("post", 14): [pa_v(12, 3)],
                ("pre", 13): [pa_dma(1024, 512)],
                ("pre", 12): [pa_qk(0, 1024, 512), pa_qk(1, 1024, 512)],
                ("post", 11): [pa_v(8, 4)],
                ("pre", 10): [pa_dma(512, 512)],
                ("pre", 9): [pa_qk(0, 512, 512)],
                ("pre", 8): [pa_qk(1, 512, 512), pa_dma(3584, 512)],
                ("pre", 7): [pa_qk(0, 3584, 512)],
                ("post", 7): [pa_v(4, 4)],
                ("pre", 6): [pa_qk(1, 3584, 512)],
                ("pre", 5): [pa_dma(0, 512), pa_dma(3072, 512)],
                ("pre", 4): [pa_qk(0, 0, 512), pa_qk(0, 3072, 512)],
                ("pre", 3): [pa_qk(1, 0, 512), pa_qk(1, 3072, 512),
                             pa_dma(2560, 512)],
                ("post", 3): [pa_v(0, 4)],
                ("pre", 2): [pa_qk(0, 2560, 512)],
                ("pre", 1): [pa_qk(1, 2560, 512), pa_dma(2048, 512)],
                ("pre", 0): [pa_qk(0, 2048, 512), pa_qk(1, 2048, 512)],
            },
            descending=True,
        )
        # attn1 runs k ASCENDING so its output slabs retire early (short tail)
        attn_batch(
            1,
            {
                ("post", 0): [pa_v(16, 4)],
                ("pre", 2): [pa_v(20, 4)],
                ("pre", 6): [pa_v(24, 4)],
                ("pre", 10): [pa_v(28, 4)],
            },
            descending=False,
        )
    nc.compile()
    return nc


_NC_CACHE = None


def make_in_maps(x: np.ndarray, Ws: np.ndarray) -> list:
    xt = np.ascontiguousarray(x.reshape(NT, E).T.astype(ml_dtypes.bfloat16))
    in_maps = []
    for c in range(NCORES):
        cols = np.concatenate(
            [
                Ws[:, c * FPC : (c + 1) * FPC],
                Ws[:, E + c * FPC : E + (c + 1) * FPC],
                Ws[:, 2 * E + c * FPC : 2 * E + (c + 1) * FPC],
            ],
            axis=1,
        ).astype(ml_dtypes.bfloat16)
        in_maps.append({"xt": xt, "wsl": np.ascontiguousarray(cols)})
    return in_maps


def assemble_out(results: list) -> np.ndarray:
    out = np.empty((B, T, H * D), np.float32)
    for c in range(NCORES):
        r = results[c]["out"].reshape(FPC, B, T)
        for b in range(B):
            out[b, :, c * FPC : (c + 1) * FPC] = r[:, b, :].T
    return out


def kernel(x: np.ndarray, Ws: np.ndarray) -> np.ndarray:
    global _NC_CACHE
    if _NC_CACHE is None:
        _NC_CACHE = build_kernel()
    nc = _NC_CACHE
    res = run_bass_kernel_spmd(
        nc, make_in_maps(x, Ws), core_ids=list(range(NCORES))
    )
    return assemble_out(res.results)


# revision 67
# speedup vs baseline: 1.0107x; 1.0107x over previous
"""Trainium2 Bass kernel for nn_MultiHeadFast (multi-head attention with
softmax over the QUERY axis).

Math (faithful to the reference):
  qkv = x @ Ws;  per (b,h):  S[q,k] = Q.K^T,  causal mask k<=q,
  P = softmax_over_q(S * T^-0.5),  out = P @ V.

Layout strategy (v4):
  * Host passes x TRANSPOSED and in bf16: xT (E, NT); device never
    transposes x.  Q^T / K^T are computed feature-on-partition; V is
    computed directly in NATURAL layout (tokens on partitions) for the PV
    stationary.  out^T is DMA'd out and transposed on the host.
  * S is computed TRANSPOSED (S^T[k, q], keys on partitions) so the
    query-axis softmax is a free-axis reduction (ACT accum during exp).
    Strips start exactly at the 128-aligned causal diagonal; the causal
    mask of the diagonal block is seeded INTO the S PSUM group by an extra
    matmul (tri_neg^T @ I), keeping the S->exp critical path PE-only.
  * QKV work is cut into small pieces (one PSUM tile each) that are
    interleaved down the attention k-loop via a deadline schedule, so the
    ACT engine never starves behind a monolithic QKV block and the PE
    stays busy (and at full clock) through the ACT-bound phase.
  * attn(b=0) runs its k-loop DESCENDING: k=15 needs only the last 128
    tokens of QKV, so attention starts ~4us in.  attn(b=1) runs ASCENDING:
    its PSUM output banks retire one-by-one (k=4j+3), spreading the output
    DMAs and shrinking the tail.
  * exp has no max-subtraction: |S*c| < 1.5.  bf16 with fp32 accumulation.

Sharding: tensor-parallel over heads.  Core c owns heads {2c, 2c+1}; no
collectives.
"""

import numpy as np
import ml_dtypes
from contextlib import ExitStack

import concourse.bass as bass
import concourse.mybir as mybir
import concourse.tile as tile
from concourse import bacc
from concourse.bass_utils import run_bass_kernel_spmd
from concourse.masks import make_identity

B, T, E = 2, 2048, 1024
H, D = 16, 64
NCORES = 8
HPC = H // NCORES            # heads per core = 2
FPC = HPC * D                # feature cols per core per Q/K/V = 128
P = 128
NT = B * T                   # 4096 tokens total
EK = E // P                  # 8 contraction blocks for QKV
KTILES = T // P              # 16 key tiles per batch
NSLAB = T // 512             # 4 query slabs per batch
DT = mybir.dt.bfloat16
F32 = mybir.dt.float32
SCALE = float(T) ** -0.5
NEG = -1e30


def build_kernel():
    nc = bacc.Bacc("TRN2", target_bir_lowering=False, debug=False)
    xt_dram = nc.dram_tensor("xt", (E, NT), DT, kind="ExternalInput")
    w_dram = nc.dram_tensor("wsl", (E, 3 * FPC), DT, kind="ExternalInput")
    out_dram = nc.dram_tensor("out", (FPC, NT), F32, kind="ExternalOutput")

    with tile.TileContext(nc) as tc, ExitStack() as ctx:
        const = ctx.enter_context(tc.tile_pool(name="const", bufs=1))
        big = ctx.enter_context(tc.tile_pool(name="big", bufs=1))
        strips = ctx.enter_context(tc.tile_pool(name="strips", bufs=4))
        small = ctx.enter_context(tc.tile_pool(name="small", bufs=8))
        outp = ctx.enter_context(tc.tile_pool(name="outp", bufs=2))
        ps = ctx.enter_context(tc.tile_pool(name="ps", bufs=2, space="PSUM"))

        # ---- input DMAs first: every engine-queue's first work is a load,
        # so transfers overlap the constant setup below ----
        wsl = big.tile([P, EK, 3 * FPC], DT, name="wsl")
        xT = big.tile([P, EK, NT], DT, name="xT")
        xt_view = xt_dram.rearrange("(eo ei) t -> ei eo t", ei=P)
        w_view = w_dram.rearrange("(eo ei) f -> ei eo f", ei=P)
        # wsl gates the first QKV matmuls: split across three DMA queues
        nc.gpsimd.dma_start(wsl[:, 0:3, :], w_view[:, 0:3, :])
        nc.sync.dma_start(xT[:, :, 1920:2048], xt_view[:, :, 1920:2048])
        nc.scalar.dma_start(wsl[:, 6:8, :], w_view[:, 6:8, :])
        nc.sync.dma_start(wsl[:, 3:6, :], w_view[:, 3:6, :])
        nc.sync.dma_start(xT[:, :, 1536:1920], xt_view[:, :, 1536:1920])

        # ---- constants ----
        zeros_bf = const.tile([P, P], DT, name="zeros_bf")
        nc.gpsimd.memset(zeros_bf[:], 0.0)
        id_bf = const.tile([P, P], DT, name="id_bf")
        make_identity(nc, id_bf)
        # tri_neg[c, m] = NEG if c < m else 0.  Used as matmul stationary to
        # seed the causal mask into the S PSUM group on the PE itself:
        # (tri_neg^T @ ext_id)[p, f] = tri_neg[f, p] = NEG iff f < p (f<128).
        tri_neg = const.tile([P, P], DT, name="tri_neg")
        nc.gpsimd.memset(tri_neg[:], 0.0)
        nc.gpsimd.affine_select(
            out=tri_neg[:],
            in_=tri_neg[:],
            compare_op=mybir.AluOpType.is_ge,
            fill=NEG,
            base=0,
            pattern=[[-1, P]],
            channel_multiplier=1,
        )
        # ext_id = [I | 0 | 0 | 0]: a 512-wide moving operand so the mask
        # seed covers the whole first S block in ONE matmul.
        ext_id = const.tile([P, 512], DT, name="ext_id")
        nc.gpsimd.memset(ext_id[:], 0.0)
        nc.vector.tensor_copy(ext_id[:, 0:P], id_bf[:])

        # ---- persistent SBUF tensors ----
        qkt = big.tile([P, 2, NT], DT, name="qkt")   # [:,0,:]=Q^T  [:,1,:]=K^T
        v_nat = big.tile([P, B * KTILES, FPC], DT, name="v_nat")

        # ---- phase-A piece builders (each = one short PSUM-tile lifetime) --
        def pa_dma(ta, w):
            def run():
                nc.sync.dma_start(xT[:, :, ta : ta + w], xt_view[:, :, ta : ta + w])
            return run

        def pa_qk(m, ta, w):
            def run():
                qk_ps = ps.tile([P, 1024], F32, tag="sps", bufs=2, name="qk_ps")
                for e in range(EK):
                    nc.tensor.matmul(
                        qk_ps[:, 0:w],
                        lhsT=wsl[:, e, m * P : (m + 1) * P],
                        rhs=xT[:, e, ta : ta + w],
                        start=(e == 0),
                        stop=(e == EK - 1),
                    )
                nc.vector.tensor_copy(qkt[:, m, ta : ta + w], qk_ps[:, 0:w])
            return run

        def pa_v(blk, nblk):
            def run():
                v_ps = ps.tile([P, 1024], F32, tag="sps", bufs=2, name="v_ps")
                for i in range(nblk):
                    t0 = (blk + i) * P
                    for e in range(EK):
                        nc.tensor.matmul(
                            v_ps[:, i * P : (i + 1) * P],
                            lhsT=xT[:, e, t0 : t0 + P],
                            rhs=wsl[:, e, 2 * FPC : 3 * FPC],
                            start=(e == 0),
                            stop=(e == EK - 1),
                        )
                nc.vector.tensor_copy(
                    v_nat[:, blk : blk + nblk, :],
                    v_ps[:, 0 : nblk * P].rearrange("p (tb f) -> p tb f", tb=nblk),
                )
            return run

        # ---- attention helpers ----
        def s_chunk(b, k, hh, strip, coff, cw):
            """S^T matmuls (+ causal-mask PSUM seed) + exp for one <=1024-wide
            chunk of a strip."""
            q0 = P * k
            kt = qkt[hh * D : (hh + 1) * D, 1, b * T + q0 : b * T + q0 + P]
            sps = ps.tile([P, 1024], F32, tag="sps", bufs=2, name="sps")
            for so in range(0, cw, 512):
                w = min(512, cw - so)
                qs = b * T + q0 + coff + so
                first = coff == 0 and so == 0
                if first:
                    # causal-mask seed for the diagonal block, one 512-wide
                    # matmul; the S matmul then accumulates on top of it.
                    nc.tensor.matmul(
                        sps[:, 0:w],
                        lhsT=tri_neg[:],
                        rhs=ext_id[:, 0:w],
                        start=True,
                        stop=False,
                    )
                nc.tensor.matmul(
                    sps[:, so : so + w],
                    lhsT=kt,
                    rhs=qkt[hh * D : (hh + 1) * D, 0, qs : qs + w],
                    start=not first,
                    stop=True,
                )
            acc = small.tile([P, 1], F32, tag="acc", name="acc")
            nc.scalar.activation(
                strip[:, coff : coff + cw],
                sps[:, :cw],
                mybir.ActivationFunctionType.Exp,
                scale=SCALE,
                accum_out=acc[:],
            )
            return acc

        def finish_head(b, k, hh, partials):
            if len(partials) == 1:
                ssum = partials[0]
            else:
                ssum = small.tile([P, 1], F32, tag="acc", name="ssum")
                nc.vector.tensor_add(ssum[:], partials[0][:], partials[1][:])
            rsum = small.tile([P, 1], F32, tag="acc", name="rsum")
            nc.vector.reciprocal(rsum[:], ssum[:])
            vp = small.tile([P, D], DT, tag="vp", name="vp")
            nc.vector.tensor_scalar_mul(
                vp[:], v_nat[:, b * KTILES + k, hh * D : (hh + 1) * D], rsum[:]
            )
            return vp

        def evac_slab(b, j, pv_ps):
            osb = outp.tile([P, 512], F32, tag="osb", name="osb")
            nc.vector.tensor_copy(osb[:], pv_ps[j][:])
            nc.sync.dma_start(
                out_dram[:, b * T + 512 * j : b * T + 512 * (j + 1)], osb[:]
            )

        def attn_batch(b, pieces_by_k, descending):
            """Attention for batch b.  pieces_by_k maps ('pre'|'post', k) ->
            piece closures emitted at the top of / mid that iteration."""
            last_k = 0 if descending else KTILES - 1
            # zero-init each PV bank just before its first writer, off the
            # prefix critical path (descending: pv[j] first written by
            # PV(4j+3), issued during iteration 4j+2; ascending: all four
            # banks first written by PV(0), issued during iteration 1).
            zinit_at = (
                {15: [3], 12: [2], 8: [1], 4: [0]}
                if descending
                else {0: [0, 1, 2, 3]}
            )

            def pv_head(k, hh, strip, vp, pv_ps):
                q0 = P * k
                j0 = k // 4
                dead = q0 - 512 * j0
                for j in range(j0, NSLAB):
                    stop = k == (0 if descending else 4 * j + 3) and hh == HPC - 1
                    if j == j0:
                        nc.tensor.matmul(
                            pv_ps[j][hh * D : (hh + 1) * D, dead:512],
                            lhsT=vp[:],
                            rhs=strip[:, 0 : 512 - dead],
                            start=False,
                            stop=stop,
                            skip_group_check=True,
                        )
                    else:
                        nc.tensor.matmul(
                            pv_ps[j][hh * D : (hh + 1) * D, :],
                            lhsT=vp[:],
                            rhs=strip[:, 512 * j - q0 : 512 * j - q0 + 512],
                            start=False,
                            stop=stop,
                            skip_group_check=True,
                        )

            pv_ps = [
                ps.tile([P, 512], F32, tag="pv", bufs=4, name=f"pv_{b}_{j}")
                for j in range(NSLAB)
            ]

            def zinit(j):
                # rhs must be initialized data (0 * NaN = NaN): wsl is loaded
                # before anything else
                nc.tensor.matmul(
                    pv_ps[j][:],
                    lhsT=zeros_bf[:],
                    rhs=wsl.rearrange("p e f -> p (e f)")[:, 0:512],
                    start=True,
                    stop=False,
                    skip_group_check=True,
                )

            ks = range(KTILES - 1, -1, -1) if descending else range(KTILES)
            prev = {}
            prev_k = None
            for k in ks:
                for piece in pieces_by_k.get(("pre", k), ()):
                    piece()
                L = T - P * k
                strip_k = {}
                parts = {0: [], 1: []}
                for hh in range(HPC):
                    strip_k[hh] = strips.tile([P, T], DT, tag="strip", name=f"s{hh}")
                coff = 0
                while coff < L:
                    cw = min(1024, L - coff)
                    for hh in range(HPC):
                        parts[hh].append(s_chunk(b, k, hh, strip_k[hh], coff, cw))
                    coff += cw
                # post-slot pieces: fill the PE while exp(k) streams on ACT
                for j in zinit_at.get(k, ()):
                    zinit(j)
                for piece in pieces_by_k.get(("post", k), ()):
                    piece()
                for hh in range(HPC):
                    vp = finish_head(b, k, hh, parts[hh])
                    if prev_k is not None:
                        pv_head(prev_k, hh, *prev[hh], pv_ps)
                    prev[hh] = (strip_k[hh], vp)
                if prev_k is not None and not descending and prev_k % 4 == 3:
                    evac_slab(b, prev_k // 4, pv_ps)
                prev_k = k
            for hh in range(HPC):
                pv_head(last_k, hh, *prev[hh], pv_ps)
            if descending:
                for j in range(NSLAB):
                    evac_slab(b, j, pv_ps)
            else:
                evac_slab(b, NSLAB - 1, pv_ps)

        # ---- program order ----
        # Short PE warmup (fits inside the initial DMA wait): ramps the PE
        # p-state so the first real QKV pieces run near full clock.
        warm_ps = ps.tile([P, 1024], F32, tag="sps", bufs=2, name="warm")
        for i in range(6):
            nc.tensor.matmul(
                warm_ps[:, 0:512],
                lhsT=zeros_bf[:],
                rhs=ext_id[:],
                start=True,
                stop=True,
                skip_group_check=True,
            )
        # prefix: just enough of slab 3 (tokens 1920-2048) for attn0's k=15
        # (its DMAs were issued at the very top)
        pa_qk(0, 1920, 128)()
        pa_qk(1, 1920, 128)()
        # attn0 runs k DESCENDING; QKV pieces stream in by deadline.
        attn_batch(
            0,
            {
                ("post", 15): [pa_v(15, 1), pa_qk(0, 1536, 384),
                               pa_qk(1, 1536, 384)],
                ("pre", 14): [pa_v(12, 3)],
                ("pre", 13): [pa_dma(1024, 512)],
                ("pre", 12): [pa_qk(0, 1024, 512), pa_qk(1, 1024, 512)],
                ("pre", 11): [pa_v(8, 4)],
                ("pre", 10): [pa_dma(512, 512)],
                ("pre", 9): [pa_qk(0, 512, 512)],
                ("pre", 8): [pa_qk(1, 512, 512), pa_dma(3584, 512)],
                ("pre", 7): [pa_v(4, 4), pa_qk(0, 3584, 512)],
                ("pre", 6): [pa_qk(1, 3584, 512)],
                ("pre", 5): [pa_dma(0, 512), pa_dma(3072, 512)],
                ("pre", 4): [pa_qk(0, 0, 512), pa_qk(0, 3072, 512)],
                ("pre", 3): [pa_qk(1, 0, 512), pa_v(0, 4), pa_qk(1, 3072, 512),
                             pa_dma(2560, 512)],
                ("pre", 2): [pa_qk(0, 2560, 512)],
                ("pre", 1): [pa_qk(1, 2560, 512), pa_dma(2048, 512)],
                ("pre", 0): [pa_qk(0, 2048, 512), pa_qk(1, 2048, 512)],
            },
            descending=True,
        )
        # attn1 runs k ASCENDING so its output slabs retire early (short tail)
        attn_batch(
            1,
            {
                ("post", 0): [pa_v(16, 4)],
                ("pre", 2): [pa_v(20, 4)],
                ("pre", 6): [pa_v(24, 4)],
                ("pre", 10): [pa_v(28, 4)],
            },
            descending=False,
        )
    nc.compile()
    return nc


_NC_CACHE = None


def make_in_maps(x: np.ndarray, Ws: np.ndarray) -> list:
    xt = np.ascontiguousarray(x.reshape(NT, E).T.astype(ml_dtypes.bfloat16))
    in_maps = []
    for c in range(NCORES):
        cols = np.concatenate(
            [
                Ws[:, c * FPC : (c + 1) * FPC],
                Ws[:, E + c * FPC : E + (c + 1) * FPC],
                Ws[:, 2 * E + c * FPC : 2 * E + (c + 1) * FPC],
            ],
            axis=1,
        ).astype(ml_dtypes.bfloat16)
        in_maps.append({"xt": xt, "wsl": np.ascontiguousarray(cols)})
    return in_maps


def assemble_out(results: list) -> np.ndarray:
    out = np.empty((B, T, H * D), np.float32)
    for c in range(NCORES):
        r = results[c]["out"].reshape(FPC, B, T)
        for b in range(B):
            out[b, :, c * FPC : (c + 1) * FPC] = r[:, b, :].T
    return out


def kernel(x: np.ndarray, Ws: np.ndarray) -> np.ndarray:
    global _NC_CACHE
    if _NC_CACHE is None:
        _NC_CACHE = build_kernel()
    nc = _NC_CACHE
    res = run_bass_kernel_spmd(
        nc, make_in_maps(x, Ws), core_ids=list(range(NCORES))
    )
    return assemble_out(res.results)
